# revision 20
# baseline (speedup 1.0000x reference)
"""Trainium2 Bass kernel for the GSAT HeteroGNN problem (8 NeuronCores).

Self-contained: hardcodes shapes/sharding; only imports the concourse
toolchain.

Strategy (dst-node sharding, SPMD over 8 cores):
  - papers split into 8 canonical chunks of 12500 (padded 12544 = 98 tiles),
    authors 8 x 6250 (padded 6272 = 49 tiles).
  - edges live on their dst's owner core, laid out host-side into 128-slot
    columns per (dst-tile, src-bank); dma_gather (int16 idx, <=32768-row
    banks) fetches source rows. Gathers use prepare_only + trigger_dma on
    4 SWDGE queues.
  - the relW (authors->papers) gathers for BOTH layers are FUSED: the
    gather table rows are [xa fp16 (256B) | h1a fp8 (256B)] so one 512B-row
    gather feeds layer-1 (slice 0:256 bitcast to fp16) and layer-2 agg
    (fp8 slices 256:384, 384:512). The table's xa half is DMA'd in from the
    input at start; the h1a fp8 half is written per-tile by the relB
    layer-1 conv; one fp8 AllGather (Shared output) replicates it.
  - relB layer-2 gathers h1p fp8 rows (256B) from an fp8 AllGather.
  - segment-mean via mask-matmul: mask[e, d] = (slot dst == d) * recip,
    precomputed HOST-side and DMA'd per group; TensorE accumulates
    aggT[feat,dst] in PSUM; psum->sbuf fp16 aggT is directly lhsT of the
    linear stage out[dst,256] = aggT.T@WlT + rootT.T@WrT (+ skip).
  - the fused-W layer-2 root term (h1p^T) is produced on-chip per tile by
    an identity-matmul transpose; the relB layer-2 root (h1a^T) comes from
    DMA transposes of the fp16 h1a chunk.
  - global mean-pool via ones-column matmuls accumulating in PSUM; final
    2-layer MLP on host in fp64.
"""
import os
import sys

try:
    import concourse  # noqa: F401
except ImportError:  # toolchain location in the grading container
    sys.path.insert(0, "/opt/trn_rl_repo")

import numpy as np
from concourse import bass, bacc, mybir, tile
from concourse import bass_utils

dt = mybir.dt

# ---------------------------------------------------------------- constants
NA, NP_, E = 50000, 100000, 300000
IN, H, OUT = 128, 256, 16
C = 8                      # cores
P = 128                    # partitions
A_CAN, P_CAN = NA // C, NP_ // C              # 6250 / 12500
A_PAD = ((A_CAN + P - 1) // P) * P            # 6272
P_PAD = ((P_CAN + P - 1) // P) * P            # 12544
NA_AG, NP_AG = C * A_PAD, C * P_PAD           # 50176 / 100352
GROUP_W = 8                # dst tiles per gather group (papers)
GROUP_B = 4                # dst tiles per gather group (authors)
NQ = 4                     # SWDGE queues for gathers
CAT = 2 * IN + H           # fused relW table row: 512 bytes (f8 units)


def _n_banks(n_rows):
    nb = (n_rows + 32767) // 32768
    return nb, (n_rows + nb - 1) // nb


class Relation:
    """Host-side uniform structure + per-core data for one edge relation."""

    def __init__(self, src, dst, n_src, src_can, src_pad, n_dst, dst_can,
                 dst_pad, recip_dst, group):
        self.n_tiles = dst_pad // P
        self.n_banks, self.bank_rows = _n_banks(C * src_pad)
        src_ag = (src // src_can) * src_pad + (src % src_can)
        dst_core = dst // dst_can
        dst_loc = dst % dst_can
        tilei = dst_loc // P
        pos = dst_loc % P
        bank = src_ag // self.bank_rows
        src_in_bank = (src_ag % self.bank_rows).astype(np.int64)

        # per (core, tile, bank) edge lists
        T, B = self.n_tiles, self.n_banks
        key = ((dst_core * T + tilei) * B + bank).astype(np.int64)
        order = np.argsort(key, kind="stable")
        key_s = key[order]
        counts = np.bincount(key_s, minlength=C * T * B).reshape(C, T, B)
        starts = np.zeros(C * T * B + 1, np.int64)
        np.cumsum(counts.ravel(), out=starts[1:])

        # uniform column counts per (tile, bank): max over cores
        self.cols_tb = np.ceil(counts.max(axis=0) / P).astype(np.int64)  # [T,B]

        # column layout: groups of GROUP tiles; within group: bank-major,
        # then tile, then that tile's columns for the bank.
        self.groups = [list(range(g, min(g + group, T)))
                       for g in range(0, T, group)]
        col = 0
        idx_off = 0
        self.tile_cols = [[] for _ in range(T)]   # global col ids per tile
        self.col_of_tb = {}
        self.ops = []        # (bank, idx_free_off, num_idxs, col_base, n_cols)
        self.group_span = []  # (col_base, n_cols) per group
        for tiles in self.groups:
            gbase = col
            for b in range(B):
                ob = col
                for t in tiles:
                    self.col_of_tb[(t, b)] = col
                    for _ in range(int(self.cols_tb[t, b])):
                        self.tile_cols[t].append(col)
                        col += 1
                nco = col - ob
                if nco:
                    self.ops.append((b, idx_off, nco * P, ob, nco))
                    idx_off += nco * P // 16
            self.group_span.append((gbase, col - gbase))
        self.total_cols = col
        self.idx_width = idx_off
        self.max_group_cols = max(n for _, n in self.group_span) if col else 0

        # per-core slot data
        self.idx16 = np.zeros((C, P, self.idx_width), np.int16)
        self.mask = np.zeros((C, P, max(col, 1) * P), np.float16)
        src_in_bank_s = src_in_bank[order]
        pos_s = pos[order]
        recip_e_s = recip_dst[dst[order]].astype(np.float32)
        for c in range(C):
            for (b, ioff, nidx, cbase, ncols) in self.ops:
                op_idx = np.zeros(nidx, np.int16)
                # tiles covered by this op, in layout order
                j0 = 0
                for t in self._op_tiles(cbase, ncols):
                    k = (c * T + t) * B + b
                    s, e2 = starts[k], starts[k + 1]
                    m = e2 - s
                    cap = int(self.cols_tb[t, b]) * P
                    assert m <= cap
                    op_idx[j0:j0 + m] = src_in_bank_s[s:e2]
                    # slot (p, col) for j within op: p=j%128, col=cbase+j//128
                    jj = np.arange(j0, j0 + m)
                    pp = jj % P
                    cc = cbase + jj // P
                    self.mask[c, pp, cc * P + pos_s[s:e2]] = \
                        recip_e_s[s:e2].astype(np.float16)
                    j0 += cap
                # wrap int16 idx: j -> [j%16, j//16], replicate to 128 parts
                w = op_idx.reshape(-1, 16).T  # [16, nidx/16]
                self.idx16[c, :, ioff:ioff + nidx // 16] = np.tile(w, (8, 1))

    def _op_tiles(self, cbase, ncols):
        out = []
        for (t, b), c0 in self.col_of_tb.items():
            if cbase <= c0 < cbase + ncols and self.cols_tb[t, b] > 0:
                out.append((t, c0))
        return [t for t, _ in sorted(out, key=lambda x: x[1])]


def _prep(inputs):
    f = lambda k: np.asarray(inputs[k], np.float32)
    x_author, x_paper = f("x_author"), f("x_paper")
    ws, wd = (np.asarray(inputs["ei_writes_src"], np.int64),
              np.asarray(inputs["ei_writes_dst"], np.int64))
    bs, bd = (np.asarray(inputs["ei_wb_src"], np.int64),
              np.asarray(inputs["ei_wb_dst"], np.int64))

    cnt_p = np.bincount(wd, minlength=NP_).astype(np.float32)
    cnt_a = np.bincount(bd, minlength=NA).astype(np.float32)
    recip_p = 1.0 / np.maximum(cnt_p, 1.0)
    recip_a = 1.0 / np.maximum(cnt_a, 1.0)

    relW = Relation(ws, wd, NA, A_CAN, A_PAD, NP_, P_CAN, P_PAD, recip_p, GROUP_W)
    relB = Relation(bs, bd, NP_, P_CAN, P_PAD, NA, A_CAN, A_PAD, recip_a, GROUP_B)

    # tables in AG layout, fp16
    xa_ag = np.zeros((NA_AG, IN), np.float16)
    xp_ag = np.zeros((NP_AG, IN), np.float16)
    for c in range(C):
        xa_ag[c * A_PAD:c * A_PAD + A_CAN] = x_author[c * A_CAN:(c + 1) * A_CAN]
        xp_ag[c * P_PAD:c * P_PAD + P_CAN] = x_paper[c * P_CAN:(c + 1) * P_CAN]

    f8 = mybir.dt.np(dt.float8e4)
    xa_bytes = xa_ag.view(np.uint8).view(f8)      # [NA_AG, 256] byte view

    # weight slab: 14 x [128, 256] fp16 (transposed: [in, out])
    wT = lambda k: f(k).T.astype(np.float16)       # [in, out]
    slabs = [wT("c1w_Wl"), wT("c1w_Wr"), wT("c1b_Wl"), wT("c1b_Wr")]
    for k in ("c2w_Wl", "c2w_Wr", "c2b_Wl", "c2b_Wr"):
        w2 = wT(k)                                  # [256, 256]
        slabs += [w2[:128], w2[128:]]
    slabs += [wT("skipA_W"), wT("skipP_W")]
    wslab = np.concatenate(slabs, axis=0)           # [14*128, 256]

    ident = np.eye(P, dtype=np.float16)

    pool_ones = np.zeros((P, 3), np.float16)
    pool_ones[:, 0] = 1.0
    pool_ones[:P_CAN - (P_PAD // P - 1) * P, 1] = 1.0   # last paper tile mask
    pool_ones[:A_CAN - (A_PAD // P - 1) * P, 2] = 1.0   # last author tile mask

    bias_nz = {k: bool(np.any(f(k))) for k in
               ("c1w_bl", "c1b_bl", "skipA_b", "skipP_b")}
    bias_p1 = np.broadcast_to(f("c1w_bl"), (P, H)).astype(np.float32).copy()
    bias_a1 = np.broadcast_to(f("c1b_bl"), (P, H)).astype(np.float32).copy()
    bias_p2 = np.broadcast_to(f("skipP_b"), (P, H)).astype(np.float32).copy()
    bias_a2 = np.broadcast_to(f("skipA_b"), (P, H)).astype(np.float32).copy()

    in_maps = []
    for c in range(C):
        in_maps.append(dict(
            xp_tab=xp_ag,
            xa_cat=xa_bytes[c * A_PAD:(c + 1) * A_PAD],
            xa_chunk=xa_ag[c * A_PAD:(c + 1) * A_PAD],
            xp_chunk=xp_ag[c * P_PAD:(c + 1) * P_PAD],
            w_idx=relW.idx16[c], w_mask=relW.mask[c],
            b_idx=relB.idx16[c], b_mask=relB.mask[c],
            wslab=wslab, ident=ident, pool_ones=pool_ones,
            bias_p1=bias_p1, bias_a1=bias_a1, bias_p2=bias_p2, bias_a2=bias_a2,
        ))
    return relW, relB, in_maps, bias_nz


def _build(relW, relB, bias_nz, debug=False):
    nc = bacc.Bacc("TRN2", target_bir_lowering=False, debug=False,
                   num_devices=C, num_swdge_queues=NQ)
    f16, f32, i16, f8e4 = dt.float16, dt.float32, dt.int16, dt.float8e4
    ein = lambda n, s, d: nc.dram_tensor(n, s, d, kind="ExternalInput")

    xp_tab = ein("xp_tab", [NP_AG, IN], f16)
    xa_cat = ein("xa_cat", [A_PAD, 2 * IN], f8e4)
    xa_chunk = ein("xa_chunk", [A_PAD, IN], f16)
    xp_chunk = ein("xp_chunk", [P_PAD, IN], f16)
    w_idx = ein("w_idx", [P, relW.idx_width], i16)
    w_mask = ein("w_mask", [P, relW.total_cols * P], f16)
    b_idx = ein("b_idx", [P, relB.idx_width], i16)
    b_mask = ein("b_mask", [P, relB.total_cols * P], f16)
    wslab = ein("wslab", [14 * P, H], f16)
    ident_in = ein("ident", [P, P], f16)
    pool_in = ein("pool_ones", [P, 3], f16)
    bias_in = {k: ein(k, [P, H], f32)
               for k in ("bias_p1", "bias_a1", "bias_p2", "bias_a2")}

    out_pool = nc.dram_tensor("out_pool", [1, 2 * H], f32, kind="ExternalOutput")
    if debug:
        dbg_h1a = nc.dram_tensor("dbg_h1a", [A_PAD, H], f16,
                                 kind="ExternalOutput")
        dbg_cat = nc.dram_tensor("dbg_cat", [A_PAD, CAT], f8e4,
                                 kind="ExternalOutput")
        dbg_h1p8 = nc.dram_tensor("dbg_h1p8", [P_PAD, H], f8e4,
                                  kind="ExternalOutput")

    W = {k: i for i, k in enumerate(
        ["c1w_Wl", "c1w_Wr", "c1b_Wl", "c1b_Wr",
         "c2w_Wl0", "c2w_Wl1", "c2w_Wr0", "c2w_Wr1",
         "c2b_Wl0", "c2b_Wl1", "c2b_Wr0", "c2b_Wr1",
         "skipA_W", "skipP_W"])}

    qsem = [nc.alloc_semaphore(f"gq{q}") for q in range(NQ)]
    qctr = [0]

    with tile.TileContext(nc) as tc:
        with tc.tile_pool(name="persist", bufs=1) as pp, \
             tc.tile_pool(name="dram", bufs=1, space="DRAM") as dp, \
             tc.tile_pool(name="work", bufs=3) as wk, \
             tc.tile_pool(name="msgs", bufs=2) as mp, \
             tc.tile_pool(name="masks", bufs=2) as mk, \
             tc.tile_pool(name="psA", bufs=4, space="PSUM") as psA, \
             tc.tile_pool(name="psL", bufs=2, space="PSUM") as psL, \
             tc.tile_pool(name="psP", bufs=1, space="PSUM") as psP:

            # ---------------- persistent loads
            wt = pp.tile([P, 14, H], f16, name="wt", tag="wt")
            nc.sync.dma_start(out=wt[:],
                              in_=wslab[:].rearrange("(s p) d -> p s d", p=P))
            ident_t = pp.tile([P, P], f16, name="ident_t", tag="ident_t")
            nc.sync.dma_start(out=ident_t[:], in_=ident_in[:])
            pool_t = pp.tile([P, 3], f16, name="pool_t", tag="pool_t")
            nc.sync.dma_start(out=pool_t[:], in_=pool_in[:])
            meta = {}
            for nm, hnd in (("w_idx", w_idx), ("b_idx", b_idx)):
                t = pp.tile(list(hnd.shape), hnd.dtype, name=nm + "_t")
                nc.sync.dma_start(out=t[:], in_=hnd[:])
                meta[nm] = t
            bias_t = {}
            for k, nz in (("bias_p1", bias_nz["c1w_bl"]),
                          ("bias_a1", bias_nz["c1b_bl"]),
                          ("bias_p2", bias_nz["skipP_b"]),
                          ("bias_a2", bias_nz["skipA_b"])):
                if nz:
                    t = pp.tile([P, H], f32, name=k + "_t")
                    nc.sync.dma_start(out=t[:], in_=bias_in[k][:])
                    bias_t[k] = t

            xaT = pp.tile([P, A_PAD], f16, name="xaT", tag="xaT")
            nc.sync.dma_start_transpose(out=xaT[:], in_=xa_chunk[:])
            xpT = pp.tile([P, P_PAD], f16, name="xpT", tag="xpT")
            nc.sync.dma_start_transpose(out=xpT[:], in_=xp_chunk[:])

            h1a_mine = dp.tile([A_PAD, H], f16, name="h1a_mine", tag="h1a_mine")
            cat_mine = dp.tile([A_PAD, CAT], f8e4, name="cat_mine",
                               tag="cat_mine")
            cat_full = dp.tile([NA_AG, CAT], f8e4, name="cat_full",
                               tag="cat_full")
            h1p8_mine = dp.tile([P_PAD, H], f8e4, name="h1p8_mine",
                                tag="h1p8_mine")
            h1p8_full = dp.tile([NP_AG, H], f8e4, name="h1p8_full",
                                tag="h1p8_full")

            # xa half of the fused table (h1a half written by relB L1 below)
            for i in range(8):
                r0, r1 = i * (A_PAD // 8), (i + 1) * (A_PAD // 8)
                nc.sync.dma_start(out=cat_mine[r0:r1, 0:2 * IN],
                                  in_=xa_cat[r0:r1, :])

            relu_f = mybir.ActivationFunctionType.Relu

            def gather_group(rel, table, elem, idx_t, cbase, ncols, msgs):
                for (b, ioff, nidx, ocb, onc) in rel.ops:
                    if not (cbase <= ocb < cbase + ncols):
                        continue
                    b0 = b * rel.bank_rows
                    b1 = min(b0 + rel.bank_rows, table.shape[0])
                    q = qctr[0] % NQ
                    qctr[0] += 1
                    nc.gpsimd.dma_gather(
                        msgs[:, ocb - cbase:ocb - cbase + onc, :],
                        table[b0:b1, :],
                        idx_t[:, ioff:ioff + nidx // 16],
                        nidx, nidx, elem, single_packet=False,
                        prepare_only=True, sem=qsem[q], queue_num=q)
                    nc.gpsimd.trigger_dma(count=None, queue_num=q)

            def load_masks(rel, mask_dram, cbase, ncols):
                maskg = mk.tile([P, rel.max_group_cols * P], f16, tag="maskg",
                                name="maskg")
                nc.sync.dma_start(
                    out=maskg[:, :ncols * P],
                    in_=mask_dram[:, cbase * P:(cbase + ncols) * P])
                return maskg

            def agg_tiles(rel, t, cbase, msgs, maskg, slices):
                """Mask-matmul aggregation for dst tile t. slices = list of
                (lo, hi, view_dtype) element ranges of the msgs rows
                (view_dtype None = native). Returns list of SBUF f16 aggT
                tiles (or None if the tile has no edges)."""
                cols = rel.tile_cols[t]
                if not cols:
                    return None
                aggs = [psA.tile([P, P], f32, tag="agg", name="agg",
                                 space="PSUM") for _ in slices]
                for i, cg in enumerate(cols):
                    cl = cg - cbase
                    for s, (lo, hi, mdt) in enumerate(slices):
                        lhsT = msgs[:, cl:cl + 1, lo:hi]
                        if mdt is not None:
                            lhsT = lhsT.bitcast(mdt)
                        nc.tensor.matmul(
                            out=aggs[s][:], lhsT=lhsT,
                            rhs=maskg[:, cl * P:(cl + 1) * P],
                            start=(i == 0), stop=(i == len(cols) - 1))
                out = []
                for a in aggs:
                    sb = wk.tile([P, P], f16, tag="aggT", name="aggT", bufs=6)
                    nc.scalar.copy(out=sb[:], in_=a[:])
                    out.append(sb)
                return out

            def linear(aggT, lhsWl, rootT, lhsWr, skipW, skipT, t, bias):
                """lin[dst, H] = sum_s aggT[s]@Wl[s] + root + skip, relu'd
                into an f16 SBUF tile."""
                lin = psL.tile([P, H], f32, tag="lin", name="lin", space="PSUM")
                first = True
                if aggT is not None:
                    for s in range(len(aggT)):
                        nc.tensor.matmul(out=lin[:], lhsT=aggT[s][:],
                                         rhs=wt[:, lhsWl[s]:lhsWl[s] + 1, :],
                                         start=first, stop=False)
                        first = False
                for s in range(len(lhsWr)):
                    nc.tensor.matmul(
                        out=lin[:], lhsT=rootT[s][:, t * P:(t + 1) * P],
                        rhs=wt[:, lhsWr[s]:lhsWr[s] + 1, :],
                        start=first, stop=(skipW is None and s == len(lhsWr) - 1))
                    first = False
                if skipW is not None:
                    nc.tensor.matmul(out=lin[:],
                                     lhsT=skipT[:, t * P:(t + 1) * P],
                                     rhs=wt[:, skipW:skipW + 1, :],
                                     start=False, stop=True)
                h = wk.tile([P, H], f16, tag="relu", name="relu", bufs=4)
                if bias is None:
                    nc.scalar.activation(out=h[:], in_=lin[:], func=relu_f)
                else:
                    tmp = wk.tile([P, H], f32, tag="btmp", name="btmp")
                    nc.vector.tensor_add(out=tmp[:], in0=lin[:], in1=bias[:])
                    nc.scalar.activation(out=h[:], in_=tmp[:], func=relu_f)
                return h

            def pool_acc(pool_ps, h, t, n_tiles, last_col):
                oc = last_col if t == n_tiles - 1 else 0
                nc.tensor.matmul(out=pool_ps[:], lhsT=pool_t[:, oc:oc + 1],
                                 rhs=h[:], start=(t == 0),
                                 stop=(t == n_tiles - 1), skip_group_check=True)

            # ================ phase 1: relB layer 1 (papers -> authors)
            for gi, tiles in enumerate(relB.groups):
                cbase, ncols = relB.group_span[gi]
                if ncols:
                    msgs = mp.tile([P, relB.max_group_cols, IN], f16,
                                   tag="msgsB1", name="msgsB1")
                    gather_group(relB, xp_tab, IN, meta["b_idx"], cbase,
                                 ncols, msgs)
                    maskg = load_masks(relB, b_mask, cbase, ncols)
                for t in tiles:
                    aggT = agg_tiles(relB, t, cbase, msgs, maskg,
                                     [(0, IN, None)])
                    h = linear(aggT, [W["c1b_Wl"]], [xaT], [W["c1b_Wr"]],
                               None, None, t, bias_t.get("bias_a1"))
                    nc.sync.dma_start(out=h1a_mine[t * P:(t + 1) * P, :],
                                      in_=h[:])
                    h8 = wk.tile([P, H], f8e4, tag="h8", name="h8", bufs=3)
                    nc.vector.tensor_copy(out=h8[:], in_=h[:])
                    nc.sync.dma_start(
                        out=cat_mine[t * P:(t + 1) * P, 2 * IN:CAT],
                        in_=h8[:])

            nc.gpsimd.collective_compute(
                "AllGather", mybir.AluOpType.bypass,
                replica_groups=[list(range(C))],
                ins=[cat_mine.opt()], outs=[cat_full.opt()])

            # h1a^T for the relB layer-2 root term (overlaps fused-W phase)
            h1aT = []
            for s in range(2):
                t = pp.tile([P, A_PAD], f16, name=f"h1aT{s}", tag=f"h1aT{s}")
                nc.scalar.dma_start_transpose(
                    out=t[:], in_=h1a_mine[:, s * P:(s + 1) * P])
                h1aT.append(t)

            # ================ phase 2: fused relW (layer 1 + layer 2)
            pool_p = psP.tile([1, H], f32, name="pool_p", tag="pool_p",
                              space="PSUM")
            pool_a = psP.tile([1, H], f32, name="pool_a", tag="pool_a",
                              space="PSUM")
            for gi, tiles in enumerate(relW.groups):
                cbase, ncols = relW.group_span[gi]
                if ncols:
                    msgs = mp.tile([P, relW.max_group_cols, CAT], f8e4,
                                   tag="msgsW", name="msgsW")
                    gather_group(relW, cat_full, CAT, meta["w_idx"], cbase,
                                 ncols, msgs)
                    maskg = load_masks(relW, w_mask, cbase, ncols)
                for t in tiles:
                    aggT = agg_tiles(
                        relW, t, cbase, msgs, maskg,
                        [(0, 2 * IN, f16), (2 * IN, 2 * IN + P, None),
                         (2 * IN + P, CAT, None)])
                    if aggT is not None:
                        a1 = [aggT[0]]
                        a2 = [aggT[1], aggT[2]]
                    else:
                        a1 = a2 = None
                    h1 = linear(a1, [W["c1w_Wl"]], [xpT], [W["c1w_Wr"]],
                                None, None, t, bias_t.get("bias_p1"))
                    h8 = wk.tile([P, H], f8e4, tag="h8", name="h8", bufs=3)
                    nc.vector.tensor_copy(out=h8[:], in_=h1[:])
                    nc.sync.dma_start(out=h1p8_mine[t * P:(t + 1) * P, :],
                                      in_=h8[:])
                    # on-chip transpose of h1 for the layer-2 root term
                    hT = []
                    for s in range(2):
                        pt = psA.tile([P, P], f32, tag="agg", name="psT",
                                      space="PSUM")
                        nc.tensor.matmul(out=pt[:],
                                         lhsT=h1[:, s * P:(s + 1) * P],
                                         rhs=ident_t[:], start=True, stop=True)
                        sb = wk.tile([P, P], f16, tag="hT", name="hT", bufs=4)
                        nc.scalar.copy(out=sb[:], in_=pt[:])
                        hT.append(sb)
                    lin2 = psL.tile([P, H], f32, tag="lin", name="lin",
                                    space="PSUM")
                    first = True
                    if a2 is not None:
                        for s, wl in enumerate([W["c2w_Wl0"], W["c2w_Wl1"]]):
                            nc.tensor.matmul(out=lin2[:], lhsT=a2[s][:],
                                             rhs=wt[:, wl:wl + 1, :],
                                             start=first, stop=False)
                            first = False
                    for s, wr in enumerate([W["c2w_Wr0"], W["c2w_Wr1"]]):
                        nc.tensor.matmul(out=lin2[:], lhsT=hT[s][:],
                                         rhs=wt[:, wr:wr + 1, :],
                                         start=first, stop=False)
                        first = False
                    nc.tensor.matmul(out=lin2[:],
                                     lhsT=xpT[:, t * P:(t + 1) * P],
                                     rhs=wt[:, W["skipP_W"]:W["skipP_W"] + 1, :],
                                     start=False, stop=True)
                    h2 = wk.tile([P, H], f16, tag="relu", name="relu", bufs=4)
                    if bias_t.get("bias_p2") is None:
                        nc.scalar.activation(out=h2[:], in_=lin2[:],
                                             func=relu_f)
                    else:
                        tmp = wk.tile([P, H], f32, tag="btmp", name="btmp")
                        nc.vector.tensor_add(out=tmp[:], in0=lin2[:],
                                             in1=bias_t["bias_p2"][:])
                        nc.scalar.activation(out=h2[:], in_=tmp[:],
                                             func=relu_f)
                    pool_acc(pool_p, h2, t, relW.n_tiles, 1)

            nc.gpsimd.collective_compute(
                "AllGather", mybir.AluOpType.bypass,
                replica_groups=[list(range(C))],
                ins=[h1p8_mine.opt()], outs=[h1p8_full.opt()])

            # ================ phase 3: relB layer 2 (papers h1 -> authors)
            for gi, tiles in enumerate(relB.groups):
                cbase, ncols = relB.group_span[gi]
                if ncols:
                    msgs = mp.tile([P, relB.max_group_cols, H], f8e4,
                                   tag="msgsB2", name="msgsB2")
                    gather_group(relB, h1p8_full, H, meta["b_idx"], cbase,
                                 ncols, msgs)
                    maskg = load_masks(relB, b_mask, cbase, ncols)
                for t in tiles:
                    aggT = agg_tiles(relB, t, cbase, msgs, maskg,
                                     [(0, P, None), (P, H, None)])
                    lin = psL.tile([P, H], f32, tag="lin", name="lin",
                                   space="PSUM")
                    first = True
                    if aggT is not None:
                        for s, wl in enumerate([W["c2b_Wl0"], W["c2b_Wl1"]]):
                            nc.tensor.matmul(out=lin[:], lhsT=aggT[s][:],
                                             rhs=wt[:, wl:wl + 1, :],
                                             start=first, stop=False)
                            first = False
                    for s, wr in enumerate([W["c2b_Wr0"], W["c2b_Wr1"]]):
                        nc.tensor.matmul(out=lin[:],
                                         lhsT=h1aT[s][:, t * P:(t + 1) * P],
                                         rhs=wt[:, wr:wr + 1, :],
                                         start=first, stop=False)
                        first = False
                    nc.tensor.matmul(out=lin[:],
                                     lhsT=xaT[:, t * P:(t + 1) * P],
                                     rhs=wt[:, W["skipA_W"]:W["skipA_W"] + 1, :],
                                     start=False, stop=True)
                    h2 = wk.tile([P, H], f16, tag="relu", name="relu", bufs=4)
                    if bias_t.get("bias_a2") is None:
                        nc.scalar.activation(out=h2[:], in_=lin[:],
                                             func=relu_f)
                    else:
                        tmp = wk.tile([P, H], f32, tag="btmp", name="btmp")
                        nc.vector.tensor_add(out=tmp[:], in0=lin[:],
                                             in1=bias_t["bias_a2"][:])
                        nc.scalar.activation(out=h2[:], in_=tmp[:],
                                             func=relu_f)
                    pool_acc(pool_a, h2, t, relB.n_tiles, 2)

            pool_sb = wk.tile([1, 2 * H], f32, tag="poolout")
            nc.vector.tensor_copy(out=pool_sb[:, 0:H], in_=pool_a[:])
            nc.vector.tensor_copy(out=pool_sb[:, H:2 * H], in_=pool_p[:])
            nc.sync.dma_start(out=out_pool[:], in_=pool_sb[:])

            if debug:
                nc.sync.dma_start(out=dbg_h1a[:], in_=h1a_mine[:])
                nc.sync.dma_start(out=dbg_cat[:], in_=cat_mine[:])
                nc.sync.dma_start(out=dbg_h1p8[:], in_=h1p8_mine[:])

    nc.compile()
    return nc


def kernel(**inputs):
    debug = bool(int(os.environ.get("GNN_DEBUG", "0")))
    trace = bool(int(os.environ.get("GNN_TRACE", "0")))
    relW, relB, in_maps, bias_nz = _prep(inputs)
    nc = _build(relW, relB, bias_nz, debug=debug)
    res = bass_utils.run_bass_kernel_spmd(
        nc, in_maps, core_ids=list(range(C)), trace=trace)
    kernel.last_results = res

    pools = np.stack([res.results[c]["out_pool"] for c in range(C)])  # [C,1,2H]
    sum_a = pools[:, 0, :H].astype(np.float64).sum(axis=0)
    sum_p = pools[:, 0, H:].astype(np.float64).sum(axis=0)
    pooled = np.concatenate([sum_a / NA, sum_p / NP_])[None, :]  # [1, 2H]
    W1 = np.asarray(inputs["cls_W1"], np.float64)
    b1 = np.asarray(inputs["cls_b1"], np.float64)
    W2 = np.asarray(inputs["cls_W2"], np.float64)
    b2 = np.asarray(inputs["cls_b2"], np.float64)
    h = np.maximum(pooled @ W1.T + b1, 0.0)
    out = h @ W2.T + b2
    return out.astype(np.float32)
